# revision 43
# baseline (speedup 1.0000x reference)
"""GQA attention kernel for Trainium2, 8 NeuronCores.

Sharding: DP=2 over batch x TP=4 over heads (8 Q heads / 2 KV heads per core).
Core c = 4*b + t handles batch b, Q heads [8t, 8t+8), KV heads [2t, 2t+2).
Each core computes a partial output (its heads' slice through Wo); the host
sums the 4 TP partials per batch.

Device-side layout: everything runs in "transposed" orientation.
Q^T/K^T ([head_dim, seq]) come from matmul(lhsT=W, rhs=x^T); scores are
computed as S^T = K^T.T @ Q^T with keys on partitions, so the exp'd
probabilities P^T feed attn@V directly as the moving operand.

fp8 DoubleRow everywhere the contraction allows it: x and all weights are
fp8 e4m3 (weights pre-scaled by 32 on the host so their mass sits in the
normal range; the scale cancels via the exp scale, the ones-column value
and the final output copy). Q/K/V projections, attnV and the output
projection run as K=256 double-pumped fp8 matmuls (half the PE cycles);
scores stay f16 (K=64 gains nothing from fp8).

Softmax: one exp per kt-PAIR on the scalar engine, reading [128,2,N] f32
scores from PSUM, writing fp8 P directly. Causal masking is done by adding
-1e9 triangle masks to the scores in PSUM (DVE) before the exp. The
denominator comes free from the attnV matmul: V carries 64 extra columns
holding the value 32.0, so o_ps rows 64:128 accumulate 32*l replicated,
and h = o * reciprocal(32 l) is two DVE ops -- no Ln/Exp, no DVE P-sum.

The emission is software-pipelined as in the baseline: window w's
attention loop interleaves window w+1's projections and window w-1's
output projection so the PE stays dense while the scalar engine drains
the exp stream.
"""

import os
import sys

for _p in ("/opt/trn_rl_repo", "/root/.axon_site/_ro/trn_rl_repo"):
    if os.path.isdir(_p) and _p not in sys.path:
        sys.path.insert(0, _p)

from collections import deque

import numpy as np
import ml_dtypes

import concourse.bass as bass
import concourse.mybir as mybir
import concourse.tile as tile

F32 = mybir.dt.float32
F16 = mybir.dt.float16
F8 = mybir.dt.float8e4
E4 = ml_dtypes.float8_e4m3
B, S, D = 2, 2048, 2048
HQ, HKV, HD = 32, 8, 64
NTP = 4          # tensor-parallel shards
HQL = HQ // NTP  # 8 local q heads
NP = HQL // 2    # 4 head pairs (j, j+4)
W = 4            # seq windows of 512
WS = S // W
DCH = D // 128   # 16 contraction chunks
SCALE = 1.0 / float(np.sqrt(HD))
EBIAS = -4.0     # exp bias; cancels in softmax, keeps fp8 P in range
WSC = 1.0        # no weight scaling needed in f16
ESCALE = SCALE
NEG = -1.0e9
DR = mybir.MatmulPerfMode.DoubleRow


def _split_sem_waits(nc, max_waits=1):
    """walrus in this container rejects >1 sem wait per instruction; move
    overflow waits onto preceding same-engine NoOps."""
    ctr = 0
    for f in nc.m.functions:
        for bb in f.blocks:
            out = []
            changed = False
            for inst in bb.instructions:
                si = getattr(inst, "sync_info", None)
                ow = list(si.on_wait) if si is not None and si.on_wait else []
                if len(ow) > max_waits:
                    changed = True
                    chunks = [ow[i:i + max_waits] for i in range(0, len(ow), max_waits)]
                    for ch in chunks[:-1]:
                        ctr += 1
                        out.append(mybir.InstNoOp(
                            name=f"{inst.name}-ws{ctr}",
                            engine=inst.engine,
                            sync_info=mybir.SyncInfo(on_wait=ch, on_update=[]),
                            bass_nofuse=True,
                            ins=[], outs=[],
                        ))
                    inst.sync_info = mybir.SyncInfo(
                        on_wait=chunks[-1],
                        on_update=list(si.on_update or []),
                    )
                out.append(inst)
            if changed:
                bb.instructions = out
    return ctr


def _build_nc(split_waits=True):
    nc = bass.Bass("TRN2", target_bir_lowering=False, debug=False, num_devices=8)

    x8_d = nc.dram_tensor("x16", [DCH, 128, S], F16, kind="ExternalInput").ap()
    wq_d = nc.dram_tensor("wq16", [128, DCH * 512], F16, kind="ExternalInput").ap()
    wk_d = nc.dram_tensor("wk16", [128, DCH * 128], F16, kind="ExternalInput").ap()
    wv_d = nc.dram_tensor("wv16", [128, DCH * 128], F16, kind="ExternalInput").ap()
    wo_d = nc.dram_tensor("wo16", [128, NP * D], F16, kind="ExternalInput").ap()
    cs_d = nc.dram_tensor("cs", [128, S], F16, kind="ExternalInput").ap()
    sn_d = nc.dram_tensor("sn", [128, S], F16, kind="ExternalInput").ap()
    rot_d = nc.dram_tensor("rot", [128, 128], F16, kind="ExternalInput").ap()
    id8_d = nc.dram_tensor("id16", [128, 128], F16, kind="ExternalInput").ap()
    m2_d = nc.dram_tensor("m2", [128, 2 * 128], F32, kind="ExternalInput").ap()
    tm_d = nc.dram_tensor("tm", [128, 128], F32, kind="ExternalInput").ap()
    out_d = nc.dram_tensor("out", [S, D], F16, kind="ExternalOutput").ap()

    mult = mybir.AluOpType.mult
    add = mybir.AluOpType.add
    Exp = mybir.ActivationFunctionType.Exp

    from contextlib import ExitStack
    with tile.TileContext(nc) as tc:
        with ExitStack() as stk:
            pool = lambda nm, bufs, **kw: stk.enter_context(
                tc.tile_pool(name=nm, bufs=bufs, **kw))
            const = pool("const", 1)
            qrp = pool("qrp", 2)
            krp = pool("krp", 4)
            vvp = pool("vvp", 4)
            vt8p = pool("vt8p", 2)
            rawp = pool("rawp", 2)
            tmpp = pool("tmpp", 3)
            pex = pool("pex", 6)
            hallp = pool("hallp", 3)
            rcp = pool("rcp", 4)
            osb = pool("osb", 4)
            pp = pool("pp", 1, space="PSUM")
            aux = pool("aux", 1, space="PSUM")
            sp = pool("sp", 2, space="PSUM")
            opp = pool("opp", 1, space="PSUM")

            # --- startup-critical DMAs first: wk + window-0's x slices, rope
            # tables, then Q-pair-0 weights; the bulk (remaining x columns,
            # Q pairs 1-3, Wv, Wo, masks) follows.
            wk_sb = const.tile([128, DCH, 128], F16, tag="wk")
            x_sb = const.tile([128, DCH, S], F16, tag="x")
            for g in range(4):
                nc.sync.dma_start(wk_sb[:, 4 * g:4 * g + 4, :],
                                  wk_d[:, g * 512:(g + 1) * 512])
                for dd in range(4 * g, 4 * g + 4):
                    nc.sync.dma_start(x_sb[:, dd, 0:WS], x8_d[dd][:, 0:WS])
            rot_sb = const.tile([128, 128], F16, tag="rot")
            nc.sync.dma_start(rot_sb[:], rot_d)
            cs_sb = const.tile([128, S], F16, tag="cs")
            nc.sync.dma_start(cs_sb[:, 0:WS], cs_d[:, 0:WS])
            sn_sb = const.tile([128, S], F16, tag="sn")
            nc.sync.dma_start(sn_sb[:, 0:WS], sn_d[:, 0:WS])
            wq_sb = const.tile([128, NP, DCH, 128], F16, tag="wq")
            nc.sync.dma_start(wq_sb[:, 0], wq_d[:, 0:DCH * 128])
            nc.sync.dma_start(cs_sb[:, WS:], cs_d[:, WS:])
            nc.sync.dma_start(sn_sb[:, WS:], sn_d[:, WS:])
            wv_sb = const.tile([128, DCH, 128], F16, tag="wv")
            nc.sync.dma_start(wv_sb[:], wv_d)
            for n in range(1, NP):
                nc.sync.dma_start(wq_sb[:, n],
                                  wq_d[:, n * DCH * 128:(n + 1) * DCH * 128])
            for dd in range(DCH):
                nc.sync.dma_start(x_sb[:, dd, WS:], x8_d[dd][:, WS:])
            id8_sb = const.tile([128, 128], F16, tag="id16")
            nc.sync.dma_start(id8_sb[:], id8_d)
            m2_sb = const.tile([128, 2, 128], F32, tag="m2")
            nc.sync.dma_start(m2_sb[:], m2_d)
            tm_sb = const.tile([128, 128], F32, tag="tm")
            nc.sync.dma_start(tm_sb[:], tm_d)
            wo_sb = const.tile([128, NP, D], F16, tag="wo")
            nc.sync.dma_start(wo_sb[:], wo_d)
            eb_sb = const.tile([128, 1], F32, tag="eb")
            nc.gpsimd.memset(eb_sb[:], EBIAS)
            # dummy exp: pull the ACT table load into the DMA wait window
            warm_sb = const.tile([128, 1], F32, tag="warm")
            nc.scalar.activation(warm_sb[:], eb_sb[:], Exp)

            kropes = []
            qropes = []
            vvs = []
            hall_by_w = {}

            def rope(ps, out_ap, wsl):
                raw = rawp.tile([128, WS], F16, tag="raw")
                nc.vector.tensor_copy(raw[:], ps[:])
                rq = aux.tile([128, WS], F32, tag="aux")
                nc.tensor.matmul(rq[:], rot_sb[:], raw[:], start=True, stop=True)
                t1 = tmpp.tile([128, WS], F16, tag="tmp")
                nc.gpsimd.tensor_tensor(t1[:], raw[:], cs_sb[:, wsl], mult)
                t2 = tmpp.tile([128, WS], F16, tag="tmp")
                nc.vector.tensor_tensor(t2[:], rq[:], sn_sb[:, wsl], mult)
                nc.gpsimd.tensor_tensor(out_ap, t1[:], t2[:], add)

            def proj_quanta(w):
                """Window w's projection stream: closures of ~850ns PE work.
                Eager part: K, Q-pair0, V (+V transpose into fp8 vv); the
                deferred part (Q pairs 1..3) streams into window w's own
                attention loop."""
                wsl = slice(w * WS, (w + 1) * WS)
                qrope = qrp.tile([128, NP, WS], F16, tag="qr")
                krope = krp.tile([128, WS], F16, tag="kr")
                vv = vvp.tile([128, 4, 2, 128], F16, tag="vv")
                qropes.append(qrope)
                kropes.append(krope)
                vvs.append(vv)
                st = {}
                quanta = []

                def chunk(key, wsel, clo):
                    def q():
                        if clo == 0:
                            st[key] = pp.tile([128, WS], F32, tag="pp",
                                              name=f"pp_{w}_{key}")
                        ps = st[key]
                        for c in range(clo, clo + 4):
                            nc.tensor.matmul(
                                ps[:], wsel(c), x_sb[:, c, wsl],
                                start=(c == 0), stop=(c == DCH - 1))
                    return q

                ropes = {('q', n): (lambda n=n: rope(st[('q', n)],
                                                     qrope[:, n, :], wsl))
                         for n in range(NP)}
                ropes['k'] = lambda: rope(st['k'], krope[:], wsl)
                for clo in range(0, DCH, 4):
                    quanta.append(chunk('k', lambda c: wk_sb[:, c, :], clo))
                for clo in range(0, DCH, 4):
                    quanta.append(chunk(('q', 0),
                                        lambda c: wq_sb[:, 0, c, :], clo))
                quanta.append(ropes['k'])
                for clo in range(0, DCH, 4):
                    quanta.append(chunk('v', lambda c: wv_sb[:, c, :], clo))
                quanta.append(ropes[('q', 0)])

                def vfin():
                    vt16 = vt8p.tile([128, WS], F16, tag="vt", name=f"vt_{w}")
                    nc.vector.tensor_copy(vt16[:], st['v'][:])
                    st['vt'] = vt16
                quanta.append(vfin)
                for i in range(4):
                    def vtr(i=i):
                        tr = aux.tile([128, 128], F16, tag="aux",
                                      name=f"tr_{w}_{i}")
                        nc.tensor.transpose(
                            tr[:], st['vt'][:, i * 128:(i + 1) * 128],
                            id8_sb[:])
                        nc.vector.tensor_copy(vv[:, i, 0, 0:64],
                                              tr[:, 0:64])
                        nc.vector.tensor_copy(vv[:, i, 1, 0:64],
                                              tr[:, 64:128])
                    quanta.append(vtr)

                def vones():
                    nc.gpsimd.memset(vv[:, :, :, 64:128], WSC)
                quanta.append(vones)

                deferred = []
                for n in range(1, NP):
                    for clo in range(0, DCH, 4):
                        deferred.append(chunk(('q', n),
                                              lambda c, n=n: wq_sb[:, n, c, :],
                                              clo))
                    deferred.append(ropes[('q', n)])
                return quanta, deferred

            def outproj_quanta(w, hall, wpool=None, wtag="aux"):
                if wpool is None:
                    wpool = aux
                quanta = []
                for stq in range(4):
                    for dwin in range(4):
                        def q(dwin=dwin, stq=stq):
                            dsl = slice(dwin * 512, (dwin + 1) * 512)
                            wops = wpool.tile([128, WS], F32, tag=wtag,
                                              name=f"wops_{w}_{dwin}_{stq}")
                            for a in range(NP):
                                nc.tensor.matmul(
                                    wops[:],
                                    hall[:, a, stq * 128:(stq + 1) * 128],
                                    wo_sb[:, a, dsl],
                                    start=(a == 0), stop=(a == NP - 1))
                            o_sb = osb.tile([128, WS], F16, tag="ou")
                            nc.vector.tensor_copy(o_sb[:], wops[:])
                            nc.sync.dma_start(
                                out_d[(w * 4 + stq) * 128:
                                      (w * 4 + stq + 1) * 128, dsl],
                                o_sb[:])
                        quanta.append(q)
                return quanta

            # prologue: window 0's eager projections run standalone.
            eager0, deferred0 = proj_quanta(0)
            for q in eager0:
                q()
            next_deferred = deferred0

            hard = deque()   # proj work: must land before next window's attn
            soft = deque()   # outproj work: free to roll across windows
            for w in range(W):
                hard.extend(next_deferred)
                next_deferred = []
                if w + 1 < W:
                    eg, df = proj_quanta(w + 1)
                    hard.extend(eg)
                    next_deferred = df
                if w >= 1:
                    soft.extend(outproj_quanta(w - 1, hall_by_w[w - 1]))
                qrope = qropes[w]
                hall = hallp.tile([128, NP, WS], F16, tag="hall",
                                  name=f"hall_{w}")
                hall_by_w[w] = hall
                npairs = 2 * w + 2
                LAG = 2
                steps_left = NP * (npairs + LAG)
                nxt_steps = NP * (2 * w + 4 + LAG) if w + 1 < W else 0
                # heads hp (PE rows 0:64) and hp+4 (rows 64:128) advance
                # together: their score matmuls alternate disjoint row-groups
                # of the PE array, so LDWEIGHTS pulls ahead and the two
                # 64-row tiles stream concurrently.
                for hp in range(NP):
                    o2 = opp.tile([128, 2, WS], F32, tag="o")
                    o_A = o2[:, 0, :]
                    o_B = o2[:, 1, :]
                    pxs = []
                    for pi in range(npairs + LAG):
                        if pi < npairs:
                            kt0 = 2 * pi
                            diag = kt0 >= 4 * w
                            qoff = max(0, kt0 - 4 * w) * 128
                            kr = kropes[kt0 // 4]
                            k0sl = slice((kt0 % 4) * 128, (kt0 % 4 + 1) * 128)
                            k1sl = slice((kt0 % 4 + 1) * 128,
                                         (kt0 % 4 + 2) * 128)
                            sA = sp.tile([128, 2, WS], F32, tag="s")
                            sB = sp.tile([128, 2, WS], F32, tag="s")
                            for ks, ksl in ((0, k0sl), (1, k1sl)):
                                nc.tensor.matmul(sA[:, ks, qoff:],
                                                 kr[0:64, ksl],
                                                 qrope[0:64, hp, qoff:],
                                                 start=True, stop=True)
                                nc.tensor.matmul(sB[:, ks, qoff:],
                                                 kr[64:128, ksl],
                                                 qrope[64:128, hp, qoff:],
                                                 start=True, stop=True)
                            if diag:
                                for s2 in (sA, sB):
                                    nc.vector.tensor_tensor(
                                        s2[:, :, qoff:qoff + 128],
                                        s2[:, :, qoff:qoff + 128],
                                        m2_sb[:], add)
                                    nc.vector.tensor_tensor(
                                        s2[:, 1, qoff + 128:qoff + 256],
                                        s2[:, 1, qoff + 128:qoff + 256],
                                        tm_sb[:], add)
                            pxA = pex.tile([128, 2, WS], F16, tag="p")
                            nc.scalar.activation(pxA[:, :, qoff:],
                                                 sA[:, :, qoff:], Exp,
                                                 scale=ESCALE, bias=eb_sb[:])
                            pxB = pex.tile([128, 2, WS], F16, tag="p")
                            nc.scalar.activation(pxB[:, :, qoff:],
                                                 sB[:, :, qoff:], Exp,
                                                 scale=ESCALE, bias=eb_sb[:])
                            pxs.append((pxA, pxB, qoff))
                        if hard:
                            npop = (len(hard) + steps_left - 1) // steps_left
                            for _ in range(min(npop, len(hard))):
                                hard.popleft()()
                        if soft:
                            npop = len(soft) // (steps_left + nxt_steps)
                            for _ in range(min(npop, len(soft))):
                                soft.popleft()()
                        steps_left -= 1
                        if pi >= LAG:
                            lpi = pi - LAG
                            pxA, pxB, lq = pxs[lpi]
                            first, last = lpi == 0, lpi == npairs - 1
                            vv = vvs[lpi // 2]
                            lk = (2 * lpi) % 4
                            for ks in range(2):
                                nc.tensor.matmul(
                                    o_A[:, lq:], vv[:, lk + ks, 0, :],
                                    pxA[:, ks, lq:],
                                    start=first and ks == 0,
                                    stop=last and ks == 1,
                                    skip_group_check=True)
                                nc.tensor.matmul(
                                    o_B[:, lq:], vv[:, lk + ks, 1, :],
                                    pxB[:, ks, lq:],
                                    start=first and ks == 0,
                                    stop=last and ks == 1,
                                    skip_group_check=True)
                    # 1/l as Exp(-Ln(l)) on the scalar engine (DVE's
                    # reciprocal is an 8-pass iterative op, ~3.4us);
                    # both heads' denominators in one Ln/Exp pass.
                    lg = rcp.tile([64, 2, WS], F32, tag="rc")
                    nc.scalar.activation(lg[:], o2[64:128, :, :],
                                         mybir.ActivationFunctionType.Ln)
                    r_sb = rcp.tile([64, 2, WS], F32, tag="rc")
                    nc.scalar.activation(r_sb[:], lg[:], Exp, scale=-1.0)
                    nc.vector.tensor_tensor(hall[0:64, hp, :],
                                            o2[0:64, 0, :], r_sb[:, 0, :],
                                            mult)
                    nc.vector.tensor_tensor(hall[64:128, hp, :],
                                            o2[0:64, 1, :], r_sb[:, 1, :],
                                            mult)
                while hard:
                    hard.popleft()()

            # epilogue: roll out remaining outproj work, then the last
            # window's output projection through the idle sp banks.
            while soft:
                soft.popleft()()
            for q in outproj_quanta(W - 1, hall_by_w[W - 1],
                                    wpool=sp, wtag="s"):
                q()

    if split_waits:
        _split_sem_waits(nc)
    return nc


_nc_cache = None


def _get_nc():
    global _nc_cache
    if _nc_cache is None:
        _nc_cache = _build_nc()
    return _nc_cache


def _host_prep(x, cos, sin, Wq, Wk, Wv, Wo):
    """Build the 8 per-core input maps."""
    f16 = np.float16
    f32 = np.float32
    cosT = np.ascontiguousarray(cos.T.astype(f16))      # [64, S]
    sinT = np.ascontiguousarray(sin.T.astype(f16))
    cs = np.concatenate([cosT, cosT], axis=0)           # [128, S]
    sn = np.concatenate([sinT, sinT], axis=0)
    R = np.zeros((128, 128), f32)
    for blk in (0, 64):
        for i in range(32):
            R[blk + i, blk + i + 32] = -1.0
            R[blk + 32 + i, blk + i] = 1.0
    rot = np.ascontiguousarray(R.T).astype(f16)         # lhsT for RQ^T = R @ Q^T
    id16 = np.eye(128, dtype=f16)
    # additive causal masks: keys on partitions, queries on cols.
    tri = np.where(np.arange(128)[:, None] > np.arange(128)[None, :],
                   np.float32(NEG), np.float32(0.0))
    m2 = np.concatenate([tri, np.full((128, 128), NEG, f32)],
                        axis=1)                          # [128, 2*128]
    m2 = np.ascontiguousarray(m2)

    def pair_perm_cols(m):                              # [D, 512] -> pair-chunked
        cols = []
        for j in range(NP):
            cols.append(m[:, j * HD:(j + 1) * HD])
            cols.append(m[:, (j + 4) * HD:(j + 5) * HD])
        return np.ascontiguousarray(np.concatenate(cols, axis=1))

    def chunk_part(m):                                  # [D, F] -> [128, DCH*F]
        f = m.shape[1]
        return np.ascontiguousarray(
            m.reshape(DCH, 128, f).transpose(1, 0, 2).reshape(128, DCH * f))

    in_maps = []
    for c in range(8):
        b, t = c // NTP, c % NTP
        xT = np.ascontiguousarray(x[b].T)               # [D, S]
        x16 = np.ascontiguousarray(
            xT.reshape(DCH, 128, S).astype(f16))
        # pair-major wq: [p][pair][chunk][128] so pair 0 is one early DMA
        wq = np.ascontiguousarray(
            pair_perm_cols(Wq[:, t * 512:(t + 1) * 512])
            .reshape(DCH, 128, NP, 128).transpose(1, 2, 0, 3)
            .reshape(128, NP * DCH * 128))
        wk = chunk_part(Wk[:, t * 128:(t + 1) * 128])
        wv = chunk_part(Wv[:, t * 128:(t + 1) * 128])
        wo = pair_perm_cols(Wo[t * 512:(t + 1) * 512, :].T).T  # [512, D]
        wo16 = np.ascontiguousarray(
            wo.reshape(NP, 128, D).transpose(1, 0, 2).reshape(128, NP * D))
        in_maps.append({
            "x16": x16,
            "wq16": wq.astype(f16),
            "wk16": wk.astype(f16),
            "wv16": wv.astype(f16),
            "wo16": wo16.astype(f16),
            "cs": cs, "sn": sn, "rot": rot, "id16": id16,
            "m2": m2, "tm": np.ascontiguousarray(tri),
        })
    return in_maps


def kernel_run(inputs, trace=False):
    from concourse.bass_utils import run_bass_kernel_spmd
    from concourse import bass_utils
    bass_utils.upload_artifacts = lambda tmpdir: "local://" + tmpdir

    if trace:
        try:
            import types
            import antenv
            if not hasattr(antenv, "axon_hooks"):
                mod = types.ModuleType("antenv.axon_hooks")
                mod._hook = None
                mod.set_axon_ntff_profile_hook = lambda h: setattr(mod, "_hook", h)
                mod.get_axon_ntff_profile_hook = lambda: mod._hook
                sys.modules["antenv.axon_hooks"] = mod
                antenv.axon_hooks = mod
                from trn_agent_boot.trn_boot import _ntff_profile_via_ctypes
                mod._hook = _ntff_profile_via_ctypes("/opt/axon/libaxon_pjrt.so")
        except Exception as e:
            print("trace hook setup failed:", e)
            trace = False
    nc = _get_nc()
    in_maps = _host_prep(inputs["x"], inputs["cos"], inputs["sin"],
                         inputs["Wq"], inputs["Wk"], inputs["Wv"], inputs["Wo"])
    res = run_bass_kernel_spmd(nc, in_maps, core_ids=list(range(8)), trace=trace)
    out = np.zeros((B, S, D), np.float32)
    for c in range(8):
        out[c // NTP] += res.results[c]["out"].astype(np.float32)
    return out, res


def kernel(**inputs) -> np.ndarray:
    out, _ = kernel_run(inputs, trace=False)
    return out


# revision 46
# speedup vs baseline: 1.0431x; 1.0431x over previous
"""GQA attention kernel for Trainium2, 8 NeuronCores.

Sharding: DP=2 over batch x TP=4 over heads (8 Q heads / 2 KV heads per core).
Core c = 4*b + t handles batch b, Q heads [8t, 8t+8), KV heads [2t, 2t+2).
Each core computes a partial output (its heads' slice through Wo); the host
sums the 4 TP partials per batch.

Device-side layout: everything runs in "transposed" orientation.
Q^T/K^T ([head_dim, seq]) come from matmul(lhsT=W, rhs=x^T); scores are
computed as S^T = K^T.T @ Q^T with keys on partitions, so the exp'd
probabilities P^T feed attn@V directly as the moving operand.

fp8 DoubleRow everywhere the contraction allows it: x and all weights are
fp8 e4m3 (weights pre-scaled by 32 on the host so their mass sits in the
normal range; the scale cancels via the exp scale, the ones-column value
and the final output copy). Q/K/V projections, attnV and the output
projection run as K=256 double-pumped fp8 matmuls (half the PE cycles);
scores stay f16 (K=64 gains nothing from fp8).

Softmax: one exp per kt-PAIR on the scalar engine, reading [128,2,N] f32
scores from PSUM, writing fp8 P directly. Causal masking is done by adding
-1e9 triangle masks to the scores in PSUM (DVE) before the exp. The
denominator comes free from the attnV matmul: V carries 64 extra columns
holding the value 32.0, so o_ps rows 64:128 accumulate 32*l replicated,
and h = o * reciprocal(32 l) is two DVE ops -- no Ln/Exp, no DVE P-sum.

The emission is software-pipelined as in the baseline: window w's
attention loop interleaves window w+1's projections and window w-1's
output projection so the PE stays dense while the scalar engine drains
the exp stream.
"""

import os
import sys

for _p in ("/opt/trn_rl_repo", "/root/.axon_site/_ro/trn_rl_repo"):
    if os.path.isdir(_p) and _p not in sys.path:
        sys.path.insert(0, _p)

from collections import deque

import numpy as np
import ml_dtypes

import concourse.bass as bass
import concourse.mybir as mybir
import concourse.tile as tile

F32 = mybir.dt.float32
F16 = mybir.dt.float16
F8 = mybir.dt.float8e4
E4 = ml_dtypes.float8_e4m3
B, S, D = 2, 2048, 2048
HQ, HKV, HD = 32, 8, 64
NTP = 4          # tensor-parallel shards
HQL = HQ // NTP  # 8 local q heads
NP = HQL // 2    # 4 head pairs (j, j+4)
W = 4            # seq windows of 512
WS = S // W
DCH = D // 128   # 16 contraction chunks
SCALE = 1.0 / float(np.sqrt(HD))
EBIAS = -4.0     # exp bias; cancels in softmax, keeps fp8 P in range
WSC = 1.0        # no weight scaling needed in f16
ESCALE = SCALE
NEG = -1.0e9
DR = mybir.MatmulPerfMode.DoubleRow


def _split_sem_waits(nc, max_waits=1):
    """walrus in this container rejects >1 sem wait per instruction; move
    overflow waits onto preceding same-engine NoOps."""
    ctr = 0
    for f in nc.m.functions:
        for bb in f.blocks:
            out = []
            changed = False
            for inst in bb.instructions:
                si = getattr(inst, "sync_info", None)
                ow = list(si.on_wait) if si is not None and si.on_wait else []
                if len(ow) > max_waits:
                    changed = True
                    chunks = [ow[i:i + max_waits] for i in range(0, len(ow), max_waits)]
                    for ch in chunks[:-1]:
                        ctr += 1
                        out.append(mybir.InstNoOp(
                            name=f"{inst.name}-ws{ctr}",
                            engine=inst.engine,
                            sync_info=mybir.SyncInfo(on_wait=ch, on_update=[]),
                            bass_nofuse=True,
                            ins=[], outs=[],
                        ))
                    inst.sync_info = mybir.SyncInfo(
                        on_wait=chunks[-1],
                        on_update=list(si.on_update or []),
                    )
                out.append(inst)
            if changed:
                bb.instructions = out
    return ctr


def _build_nc(split_waits=True):
    nc = bass.Bass("TRN2", target_bir_lowering=False, debug=False, num_devices=8)

    x8_d = nc.dram_tensor("x16", [DCH, 128, S], F16, kind="ExternalInput").ap()
    wq_d = nc.dram_tensor("wq16", [128, DCH * 512], F16, kind="ExternalInput").ap()
    wk_d = nc.dram_tensor("wk16", [128, DCH * 128], F16, kind="ExternalInput").ap()
    wv_d = nc.dram_tensor("wv16", [128, DCH * 128], F16, kind="ExternalInput").ap()
    wo_d = nc.dram_tensor("wo16", [128, NP * D], F16, kind="ExternalInput").ap()
    cs_d = nc.dram_tensor("cs", [128, S], F16, kind="ExternalInput").ap()
    sn_d = nc.dram_tensor("sn", [128, S], F16, kind="ExternalInput").ap()
    rot_d = nc.dram_tensor("rot", [128, 128], F16, kind="ExternalInput").ap()
    id8_d = nc.dram_tensor("id16", [128, 128], F16, kind="ExternalInput").ap()
    m2_d = nc.dram_tensor("m2", [128, 2 * 128], F32, kind="ExternalInput").ap()
    tm_d = nc.dram_tensor("tm", [128, 128], F32, kind="ExternalInput").ap()
    out_d = nc.dram_tensor("out", [S, D], F16, kind="ExternalOutput").ap()

    mult = mybir.AluOpType.mult
    add = mybir.AluOpType.add
    Exp = mybir.ActivationFunctionType.Exp

    from contextlib import ExitStack
    with tile.TileContext(nc) as tc:
        with ExitStack() as stk:
            pool = lambda nm, bufs, **kw: stk.enter_context(
                tc.tile_pool(name=nm, bufs=bufs, **kw))
            const = pool("const", 1)
            qrp = pool("qrp", 2)
            krp = pool("krp", 4)
            vvp = pool("vvp", 4)
            vt8p = pool("vt8p", 2)
            rawp = pool("rawp", 2)
            tmpp = pool("tmpp", 3)
            pex = pool("pex", 8)
            hallp = pool("hallp", 3)
            rcp = pool("rcp", 4)
            osb = pool("osb", 4)
            pp = pool("pp", 1, space="PSUM")
            aux = pool("aux", 1, space="PSUM")
            sp = pool("sp", 2, space="PSUM")
            opp = pool("opp", 1, space="PSUM")

            # --- startup-critical DMAs first: wk + window-0's x slices, rope
            # tables, then Q-pair-0 weights; the bulk (remaining x columns,
            # Q pairs 1-3, Wv, Wo, masks) follows.
            wk_sb = const.tile([128, DCH, 128], F16, tag="wk")
            x_sb = const.tile([128, DCH, S], F16, tag="x")
            for g in range(4):
                nc.sync.dma_start(wk_sb[:, 4 * g:4 * g + 4, :],
                                  wk_d[:, g * 512:(g + 1) * 512])
                for dd in range(4 * g, 4 * g + 4):
                    nc.sync.dma_start(x_sb[:, dd, 0:WS], x8_d[dd][:, 0:WS])
            rot_sb = const.tile([128, 128], F16, tag="rot")
            nc.sync.dma_start(rot_sb[:], rot_d)
            cs_sb = const.tile([128, S], F16, tag="cs")
            nc.sync.dma_start(cs_sb[:, 0:WS], cs_d[:, 0:WS])
            sn_sb = const.tile([128, S], F16, tag="sn")
            nc.sync.dma_start(sn_sb[:, 0:WS], sn_d[:, 0:WS])
            wq_sb = const.tile([128, NP, DCH, 128], F16, tag="wq")
            nc.sync.dma_start(wq_sb[:, 0], wq_d[:, 0:DCH * 128])
            nc.sync.dma_start(cs_sb[:, WS:], cs_d[:, WS:])
            nc.sync.dma_start(sn_sb[:, WS:], sn_d[:, WS:])
            wv_sb = const.tile([128, DCH, 128], F16, tag="wv")
            nc.sync.dma_start(wv_sb[:], wv_d)
            for n in range(1, NP):
                nc.sync.dma_start(wq_sb[:, n],
                                  wq_d[:, n * DCH * 128:(n + 1) * DCH * 128])
            for dd in range(DCH):
                nc.sync.dma_start(x_sb[:, dd, WS:], x8_d[dd][:, WS:])
            id8_sb = const.tile([128, 128], F16, tag="id16")
            nc.sync.dma_start(id8_sb[:], id8_d)
            m2_sb = const.tile([128, 2, 128], F32, tag="m2")
            nc.sync.dma_start(m2_sb[:], m2_d)
            tm_sb = const.tile([128, 128], F32, tag="tm")
            nc.sync.dma_start(tm_sb[:], tm_d)
            wo_sb = const.tile([128, NP, D], F16, tag="wo")
            nc.sync.dma_start(wo_sb[:], wo_d)
            eb_sb = const.tile([128, 1], F32, tag="eb")
            nc.gpsimd.memset(eb_sb[:], EBIAS)
            # dummy exp: pull the ACT table load into the DMA wait window
            warm_sb = const.tile([128, 1], F32, tag="warm")
            nc.scalar.activation(warm_sb[:], eb_sb[:], Exp)

            kropes = []
            qropes = []
            vvs = []
            hall_by_w = {}

            def rope(ps, out_ap, wsl):
                raw = rawp.tile([128, WS], F16, tag="raw")
                nc.vector.tensor_copy(raw[:], ps[:])
                rq = aux.tile([128, WS], F32, tag="aux")
                nc.tensor.matmul(rq[:], rot_sb[:], raw[:], start=True, stop=True)
                t1 = tmpp.tile([128, WS], F16, tag="tmp")
                nc.gpsimd.tensor_tensor(t1[:], raw[:], cs_sb[:, wsl], mult)
                t2 = tmpp.tile([128, WS], F16, tag="tmp")
                nc.vector.tensor_tensor(t2[:], rq[:], sn_sb[:, wsl], mult)
                nc.gpsimd.tensor_tensor(out_ap, t1[:], t2[:], add)

            def proj_quanta(w):
                """Window w's projection stream: closures of ~850ns PE work.
                Eager part: K, Q-pair0, V (+V transpose into fp8 vv); the
                deferred part (Q pairs 1..3) streams into window w's own
                attention loop."""
                wsl = slice(w * WS, (w + 1) * WS)
                qrope = qrp.tile([128, NP, WS], F16, tag="qr")
                krope = krp.tile([128, WS], F16, tag="kr")
                vv = vvp.tile([128, 4, 2, 128], F16, tag="vv")
                qropes.append(qrope)
                kropes.append(krope)
                vvs.append(vv)
                st = {}
                quanta = []

                def chunk(key, wsel, clo):
                    def q():
                        if clo == 0:
                            st[key] = pp.tile([128, WS], F32, tag="pp",
                                              name=f"pp_{w}_{key}")
                        ps = st[key]
                        for c in range(clo, clo + 4):
                            nc.tensor.matmul(
                                ps[:], wsel(c), x_sb[:, c, wsl],
                                start=(c == 0), stop=(c == DCH - 1))
                    return q

                ropes = {('q', n): (lambda n=n: rope(st[('q', n)],
                                                     qrope[:, n, :], wsl))
                         for n in range(NP)}
                ropes['k'] = lambda: rope(st['k'], krope[:], wsl)
                for clo in range(0, DCH, 4):
                    quanta.append(chunk('k', lambda c: wk_sb[:, c, :], clo))
                for clo in range(0, DCH, 4):
                    quanta.append(chunk(('q', 0),
                                        lambda c: wq_sb[:, 0, c, :], clo))
                quanta.append(ropes['k'])
                for clo in range(0, DCH, 4):
                    quanta.append(chunk('v', lambda c: wv_sb[:, c, :], clo))
                quanta.append(ropes[('q', 0)])

                def vfin():
                    vt16 = vt8p.tile([128, WS], F16, tag="vt", name=f"vt_{w}")
                    nc.vector.tensor_copy(vt16[:], st['v'][:])
                    st['vt'] = vt16
                quanta.append(vfin)
                for i in range(4):
                    def vtr(i=i):
                        tr = aux.tile([128, 128], F16, tag="aux",
                                      name=f"tr_{w}_{i}")
                        nc.tensor.transpose(
                            tr[:], st['vt'][:, i * 128:(i + 1) * 128],
                            id8_sb[:])
                        nc.vector.tensor_copy(vv[:, i, 0, 0:64],
                                              tr[:, 0:64])
                        nc.vector.tensor_copy(vv[:, i, 1, 0:64],
                                              tr[:, 64:128])
                    quanta.append(vtr)

                def vones():
                    nc.gpsimd.memset(vv[:, :, :, 64:128], WSC)
                quanta.append(vones)

                deferred = []
                for n in range(1, NP):
                    for clo in range(0, DCH, 4):
                        deferred.append(chunk(('q', n),
                                              lambda c, n=n: wq_sb[:, n, c, :],
                                              clo))
                    deferred.append(ropes[('q', n)])
                return quanta, deferred

            def outproj_quanta(w, hall, wpool=None, wtag="aux"):
                if wpool is None:
                    wpool = aux
                quanta = []
                for stq in range(4):
                    for dwin in range(4):
                        def q(dwin=dwin, stq=stq):
                            dsl = slice(dwin * 512, (dwin + 1) * 512)
                            wops = wpool.tile([128, WS], F32, tag=wtag,
                                              name=f"wops_{w}_{dwin}_{stq}")
                            for a in range(NP):
                                nc.tensor.matmul(
                                    wops[:],
                                    hall[:, a, stq * 128:(stq + 1) * 128],
                                    wo_sb[:, a, dsl],
                                    start=(a == 0), stop=(a == NP - 1))
                            o_sb = osb.tile([128, WS], F16, tag="ou")
                            nc.vector.tensor_copy(o_sb[:], wops[:])
                            nc.sync.dma_start(
                                out_d[(w * 4 + stq) * 128:
                                      (w * 4 + stq + 1) * 128, dsl],
                                o_sb[:])
                        quanta.append(q)
                return quanta

            # prologue: window 0's eager projections run standalone.
            eager0, deferred0 = proj_quanta(0)
            for q in eager0:
                q()
            next_deferred = deferred0

            hard = deque()   # proj work: must land before next window's attn
            soft = deque()   # outproj work: free to roll across windows
            for w in range(W):
                hard.extend(next_deferred)
                next_deferred = []
                if w + 1 < W:
                    eg, df = proj_quanta(w + 1)
                    hard.extend(eg)
                    next_deferred = df
                if w >= 1:
                    soft.extend(outproj_quanta(w - 1, hall_by_w[w - 1]))
                qrope = qropes[w]
                hall = hallp.tile([128, NP, WS], F16, tag="hall",
                                  name=f"hall_{w}")
                hall_by_w[w] = hall
                npairs = 2 * w + 2
                LAG = 3
                steps_left = NP * (npairs + LAG)
                nxt_steps = NP * (2 * w + 4 + LAG) if w + 1 < W else 0
                # heads hp (PE rows 0:64) and hp+4 (rows 64:128) advance
                # together: their score matmuls alternate disjoint row-groups
                # of the PE array, so LDWEIGHTS pulls ahead and the two
                # 64-row tiles stream concurrently.
                for hp in range(NP):
                    o2 = opp.tile([128, 2, WS], F32, tag="o")
                    o_A = o2[:, 0, :]
                    o_B = o2[:, 1, :]
                    pxs = []
                    for pi in range(npairs + LAG):
                        if pi < npairs:
                            kt0 = 2 * pi
                            diag = kt0 >= 4 * w
                            qoff = max(0, kt0 - 4 * w) * 128
                            qoff1 = qoff + 128 if diag else qoff
                            kr = kropes[kt0 // 4]
                            k0sl = slice((kt0 % 4) * 128, (kt0 % 4 + 1) * 128)
                            k1sl = slice((kt0 % 4 + 1) * 128,
                                         (kt0 % 4 + 2) * 128)
                            sA = sp.tile([128, 2, WS], F32, tag="s")
                            sB = sp.tile([128, 2, WS], F32, tag="s")
                            for ks, ksl, qo in ((0, k0sl, qoff),
                                                (1, k1sl, qoff1)):
                                nc.tensor.matmul(sA[:, ks, qo:],
                                                 kr[0:64, ksl],
                                                 qrope[0:64, hp, qo:],
                                                 start=True, stop=True)
                                nc.tensor.matmul(sB[:, ks, qo:],
                                                 kr[64:128, ksl],
                                                 qrope[64:128, hp, qo:],
                                                 start=True, stop=True)
                            if diag:
                                for s2 in (sA, sB):
                                    nc.vector.tensor_tensor(
                                        s2[:, 0, qoff:qoff + 128],
                                        s2[:, 0, qoff:qoff + 128],
                                        tm_sb[:], add)
                                    nc.vector.tensor_tensor(
                                        s2[:, 1, qoff1:qoff1 + 128],
                                        s2[:, 1, qoff1:qoff1 + 128],
                                        tm_sb[:], add)
                            # the exp covers [qoff:] for both kt halves; for
                            # diag pairs kt1's [qoff:qoff1) slice is stale
                            # PSUM whose exp output is never consumed (attnV
                            # reads kt1 from qoff1).
                            pxA = pex.tile([128, 2, WS], F16, tag="p")
                            nc.scalar.activation(pxA[:, :, qoff:],
                                                 sA[:, :, qoff:], Exp,
                                                 scale=ESCALE, bias=eb_sb[:])
                            pxB = pex.tile([128, 2, WS], F16, tag="p")
                            nc.scalar.activation(pxB[:, :, qoff:],
                                                 sB[:, :, qoff:], Exp,
                                                 scale=ESCALE, bias=eb_sb[:])
                            pxs.append((pxA, pxB, qoff, qoff1))
                        if hard:
                            npop = (len(hard) + steps_left - 1) // steps_left
                            for _ in range(min(npop, len(hard))):
                                hard.popleft()()
                        if soft:
                            npop = len(soft) // (steps_left + nxt_steps)
                            for _ in range(min(npop, len(soft))):
                                soft.popleft()()
                        steps_left -= 1
                        if pi >= LAG:
                            lpi = pi - LAG
                            pxA, pxB, lq0, lq1 = pxs[lpi]
                            first, last = lpi == 0, lpi == npairs - 1
                            vv = vvs[lpi // 2]
                            lk = (2 * lpi) % 4
                            for ks, lqv in ((0, lq0), (1, lq1)):
                                nc.tensor.matmul(
                                    o_A[:, lqv:], vv[:, lk + ks, 0, :],
                                    pxA[:, ks, lqv:],
                                    start=first and ks == 0,
                                    stop=last and ks == 1,
                                    skip_group_check=True)
                                nc.tensor.matmul(
                                    o_B[:, lqv:], vv[:, lk + ks, 1, :],
                                    pxB[:, ks, lqv:],
                                    start=first and ks == 0,
                                    stop=last and ks == 1,
                                    skip_group_check=True)
                    # 1/l as Exp(-Ln(l)) on the scalar engine (DVE's
                    # reciprocal is an 8-pass iterative op, ~3.4us);
                    # both heads' denominators in one Ln/Exp pass.
                    lg = rcp.tile([64, 2, WS], F32, tag="rc")
                    nc.scalar.activation(lg[:], o2[64:128, :, :],
                                         mybir.ActivationFunctionType.Ln)
                    r_sb = rcp.tile([64, 2, WS], F32, tag="rc")
                    nc.scalar.activation(r_sb[:], lg[:], Exp, scale=-1.0)
                    nc.vector.tensor_tensor(hall[0:64, hp, :],
                                            o2[0:64, 0, :], r_sb[:, 0, :],
                                            mult)
                    nc.vector.tensor_tensor(hall[64:128, hp, :],
                                            o2[0:64, 1, :], r_sb[:, 1, :],
                                            mult)
                while hard:
                    hard.popleft()()

            # epilogue: roll out remaining outproj work, then the last
            # window's output projection through the idle sp banks.
            while soft:
                soft.popleft()()
            for q in outproj_quanta(W - 1, hall_by_w[W - 1],
                                    wpool=sp, wtag="s"):
                q()

    if split_waits:
        _split_sem_waits(nc)
    return nc


_nc_cache = None


def _get_nc():
    global _nc_cache
    if _nc_cache is None:
        _nc_cache = _build_nc()
    return _nc_cache


def _host_prep(x, cos, sin, Wq, Wk, Wv, Wo):
    """Build the 8 per-core input maps."""
    f16 = np.float16
    f32 = np.float32
    cosT = np.ascontiguousarray(cos.T.astype(f16))      # [64, S]
    sinT = np.ascontiguousarray(sin.T.astype(f16))
    cs = np.concatenate([cosT, cosT], axis=0)           # [128, S]
    sn = np.concatenate([sinT, sinT], axis=0)
    R = np.zeros((128, 128), f32)
    for blk in (0, 64):
        for i in range(32):
            R[blk + i, blk + i + 32] = -1.0
            R[blk + 32 + i, blk + i] = 1.0
    rot = np.ascontiguousarray(R.T).astype(f16)         # lhsT for RQ^T = R @ Q^T
    id16 = np.eye(128, dtype=f16)
    # additive causal masks: keys on partitions, queries on cols.
    tri = np.where(np.arange(128)[:, None] > np.arange(128)[None, :],
                   np.float32(NEG), np.float32(0.0))
    m2 = np.concatenate([tri, np.full((128, 128), NEG, f32)],
                        axis=1)                          # [128, 2*128]
    m2 = np.ascontiguousarray(m2)

    def pair_perm_cols(m):                              # [D, 512] -> pair-chunked
        cols = []
        for j in range(NP):
            cols.append(m[:, j * HD:(j + 1) * HD])
            cols.append(m[:, (j + 4) * HD:(j + 5) * HD])
        return np.ascontiguousarray(np.concatenate(cols, axis=1))

    def chunk_part(m):                                  # [D, F] -> [128, DCH*F]
        f = m.shape[1]
        return np.ascontiguousarray(
            m.reshape(DCH, 128, f).transpose(1, 0, 2).reshape(128, DCH * f))

    in_maps = []
    for c in range(8):
        b, t = c // NTP, c % NTP
        xT = np.ascontiguousarray(x[b].T)               # [D, S]
        x16 = np.ascontiguousarray(
            xT.reshape(DCH, 128, S).astype(f16))
        # pair-major wq: [p][pair][chunk][128] so pair 0 is one early DMA
        wq = np.ascontiguousarray(
            pair_perm_cols(Wq[:, t * 512:(t + 1) * 512])
            .reshape(DCH, 128, NP, 128).transpose(1, 2, 0, 3)
            .reshape(128, NP * DCH * 128))
        wk = chunk_part(Wk[:, t * 128:(t + 1) * 128])
        wv = chunk_part(Wv[:, t * 128:(t + 1) * 128])
        wo = pair_perm_cols(Wo[t * 512:(t + 1) * 512, :].T).T  # [512, D]
        wo16 = np.ascontiguousarray(
            wo.reshape(NP, 128, D).transpose(1, 0, 2).reshape(128, NP * D))
        in_maps.append({
            "x16": x16,
            "wq16": wq.astype(f16),
            "wk16": wk.astype(f16),
            "wv16": wv.astype(f16),
            "wo16": wo16.astype(f16),
            "cs": cs, "sn": sn, "rot": rot, "id16": id16,
            "m2": m2, "tm": np.ascontiguousarray(tri),
        })
    return in_maps


def kernel_run(inputs, trace=False):
    from concourse.bass_utils import run_bass_kernel_spmd
    from concourse import bass_utils
    bass_utils.upload_artifacts = lambda tmpdir: "local://" + tmpdir

    if trace:
        try:
            import types
            import antenv
            if not hasattr(antenv, "axon_hooks"):
                mod = types.ModuleType("antenv.axon_hooks")
                mod._hook = None
                mod.set_axon_ntff_profile_hook = lambda h: setattr(mod, "_hook", h)
                mod.get_axon_ntff_profile_hook = lambda: mod._hook
                sys.modules["antenv.axon_hooks"] = mod
                antenv.axon_hooks = mod
                from trn_agent_boot.trn_boot import _ntff_profile_via_ctypes
                mod._hook = _ntff_profile_via_ctypes("/opt/axon/libaxon_pjrt.so")
        except Exception as e:
            print("trace hook setup failed:", e)
            trace = False
    nc = _get_nc()
    in_maps = _host_prep(inputs["x"], inputs["cos"], inputs["sin"],
                         inputs["Wq"], inputs["Wk"], inputs["Wv"], inputs["Wo"])
    res = run_bass_kernel_spmd(nc, in_maps, core_ids=list(range(8)), trace=trace)
    out = np.zeros((B, S, D), np.float32)
    for c in range(8):
        out[c // NTP] += res.results[c]["out"].astype(np.float32)
    return out, res


def kernel(**inputs) -> np.ndarray:
    out, _ = kernel_run(inputs, trace=False)
    return out
